# revision 53
# baseline (speedup 1.0000x reference)
"""ChunkwiseDeltaAttention Trainium2 Bass kernel (v2).

Math (per reference):
  q = hs @ q_w.T ; k = ... ; v = ... (heads: 16 x 128; biases are zero
  for the graded inputs -- checked on host, folded paths only built when
  nonzero)
  beta = softplus(hs @ b_w.T) = ln(1 + e^x)
  qn, kn = l2norm per head (the /sqrt(d) pre-scale cancels)
  per 64-chunk: out = tril(qn @ kn^T) @ (beta*v)   (decay==1 on the tri)
  y = (out * silu(hs @ og_w.T)) @ o_w.T

Sharding: token-parallel -- 8 contiguous 2048-token slices, weights
replicated. All matmuls bf16 with f32 PSUM.

Key structure vs v1 (what makes it fast):
  * q/k are projected DIRECTLY transposed (w as stationary, x^T as
    moving) -> [d, tok] tiles with no PE transposes and no token-major
    epilogue chain. Norms are computed with per-tile ones-matmuls
    (sum over d partitions -> [tok,1]) and rsqrt = exp(-0.5*ln(n2)).
    The q-norm folds into the silu gate, the k-norm into beta, so the
    attention math never multiplies by 1/||.|| explicitly.
  * ACT-table usage kept cheap: softplus = ln(1+e^x), rsqrt =
    exp(-0.5*ln); the gate uses the ACT Silu table (square/copy are in
    every set, so only ~2 table switches per head-group, off-critical).
    Never use DVE reciprocal on large tiles (3.4us per [128,512]).
  * Attention (S^T -> mask -> O -> gate) is software-pipelined: its
    PE micro-batches are interleaved between the NEXT projection's
    psum tiles so the PE never waits on the DVE/ACT epilogues.
  * G^T stays in SBUF (no DRAM round-trip before o_proj); o_proj weights
    stream in during attention; y is written bf16.
  * Norm matmuls are emitted one psum-tile late so they never stall PE.
"""

import numpy as np
import ml_dtypes

import concourse.bass as bass
import concourse.mybir as mybir
from concourse import bacc
from concourse.tile import TileContext

BF16 = mybir.dt.bfloat16
F32 = mybir.dt.float32
NPBF = ml_dtypes.bfloat16
AF = mybir.ActivationFunctionType
ALU = mybir.AluOpType

NH = 16      # heads
D = 128      # head dim
CHUNK = 64
H = 2048     # hidden size
N_CORES = 8
KH = H // 128          # hidden k-tiles (16)
COLS = 4 * H + NH      # wcat columns: q|k|v|og|b = 8208
B_OFF = 4 * H


def build_nc(T=2048, num_devices=N_CORES, has_qk_bias=False,
             has_vog_bias=False, has_b_bias=False, has_o_bias=False):
    """Per-core Bass program for a T-token slice."""
    NG = 2 if T >= 2048 else 1
    TG = T // NG           # tokens per group
    MG = TG // 128         # 128-token tiles per group
    NT = TG // 512         # 512-token tiles per group

    nc = bacc.Bacc("TRN2", target_bir_lowering=False, debug=False,
                   num_devices=num_devices)

    # xt3: per-512-token tile-contiguous activations [n, p, (k c)]
    # wblk: pre-tiled weight blocks [q0-3 | k0-3 | v0-3 | og0-3 | ow0-3],
    #       each [128, KH*512] partition-contiguous -> 128 x 16KB descriptors
    NB = T // 512
    xt3 = nc.dram_tensor("xt3", [NB, 128, KH * 512], BF16,
                         kind="ExternalInput")
    wblk = nc.dram_tensor("wblk", [20, 128, KH * 512], BF16,
                          kind="ExternalInput")
    wbt = nc.dram_tensor("wbt", [128, KH * NH], BF16, kind="ExternalInput")
    maskt = nc.dram_tensor("maskt", [128, 128], F32, kind="ExternalInput")
    ident = nc.dram_tensor("ident", [128, 128], BF16, kind="ExternalInput")
    onesd = nc.dram_tensor("onesd", [128, 1], BF16, kind="ExternalInput")
    if has_qk_bias:
        qkb = nc.dram_tensor("qkb", [128, 2 * NH], F32, kind="ExternalInput")
    if has_vog_bias:
        vob = nc.dram_tensor("vob", [1, 2 * H], F32, kind="ExternalInput")
    if has_b_bias:
        bbd = nc.dram_tensor("bbd", [1, NH], F32, kind="ExternalInput")
    if has_o_bias:
        obd = nc.dram_tensor("obd", [1, H], F32, kind="ExternalInput")

    y = nc.dram_tensor("y", [T, H], BF16, kind="ExternalOutput")

    import contextlib
    with TileContext(nc) as tc:
        with contextlib.ExitStack() as _st:
            singles = _st.enter_context(tc.tile_pool(name="singles", bufs=1))
            xtp = _st.enter_context(tc.tile_pool(name="xtp", bufs=1))
            wpool = _st.enter_context(tc.tile_pool(name="wpool", bufs=3))
            qkp = _st.enter_context(tc.tile_pool(name="qkp", bufs=2))
            vgp = _st.enter_context(tc.tile_pool(name="vgp", bufs=2))
            gtp = _st.enter_context(tc.tile_pool(name="gtp", bufs=1))
            sqp = _st.enter_context(tc.tile_pool(name="sqp", bufs=3))
            atp = _st.enter_context(tc.tile_pool(name="atp", bufs=9))
            gop = _st.enter_context(tc.tile_pool(name="gop", bufs=3))
            work = _st.enter_context(tc.tile_pool(name="work", bufs=4))
            yp = _st.enter_context(tc.tile_pool(name="yp", bufs=2))
            small = _st.enter_context(tc.tile_pool(name="small", bufs=2))
            psA = _st.enter_context(
                tc.tile_pool(name="psA", bufs=3, space="PSUM"))
            psSp = _st.enter_context(
                tc.tile_pool(name="psS", bufs=1, space="PSUM"))
            psTp = _st.enter_context(
                tc.tile_pool(name="psT", bufs=2, space="PSUM"))
            psMp = _st.enter_context(
                tc.tile_pool(name="psM", bufs=1, space="PSUM"))
            mask_sb = singles.tile([128, 128], F32)
            nc.sync.dma_start(out=mask_sb, in_=maskt[:, :])
            id_sb = singles.tile([128, 128], BF16)
            nc.sync.dma_start(out=id_sb, in_=ident[:, :])
            ones_sb = singles.tile([128, 1], BF16)
            nc.sync.dma_start(out=ones_sb, in_=onesd[:, :])
            wb_sb = singles.tile([128, KH, NH], BF16)
            nc.sync.dma_start(
                out=wb_sb,
                in_=wbt[:, :].rearrange("p (k c) -> p k c", k=KH))
            if has_qk_bias:
                qkb_sb = singles.tile([128, 2 * NH], F32)
                nc.sync.dma_start(out=qkb_sb, in_=qkb[:, :])
            if has_b_bias:
                bb_sb = singles.tile([128, NH], F32)
                nc.gpsimd.dma_start(out=bb_sb, in_=bass.AP(
                    tensor=bbd.ap().tensor, offset=0, ap=[[0, 128], [1, NH]]))

            def load_w(bi, tag="w"):
                t = wpool.tile([128, KH, 512], BF16, tag=tag)
                nc.sync.dma_start(
                    out=t, in_=wblk[bi, :, :].rearrange(
                        "p (k c) -> p k c", k=KH))
                return t

            def load_xt(xt_sb, g, n):
                nc.sync.dma_start(
                    out=xt_sb[:, :, n * 512:(n + 1) * 512],
                    in_=xt3[g * NT + n, :, :].rearrange(
                        "p (k c) -> p k c", k=KH))

            w_pre = {}
            for g in range(NG):
                xt_sb = xtp.tile([128, KH, TG], BF16, tag="xt")
                if g == 0:
                    # first group: interleave weight + xt streams so the PE
                    # can start early and never starves through j0
                    # interleave quarter-granular w_q / xt streams so the
                    # first psum tile's k=0..3 matmuls start ASAP
                    wq = wpool.tile([128, KH, 512], BF16, tag="w")
                    KQ = KH // 4
                    for qtr in range(4):
                        lo, hi = qtr * KQ, (qtr + 1) * KQ
                        nc.sync.dma_start(
                            out=wq[:, lo:hi, :],
                            in_=wblk[0, :, lo * 512:hi * 512].rearrange(
                                "p (k c) -> p k c", k=KQ))
                        nc.sync.dma_start(
                            out=xt_sb[:, lo:hi, 0:512],
                            in_=xt3[0, :, lo * 512:hi * 512].rearrange(
                                "p (k c) -> p k c", k=KQ))
                    w_pre["q"] = wq
                    for n in range(1, NT):
                        load_xt(xt_sb, g, n)
                    w_pre["k"] = load_w(4)
                else:
                    for n in range(NT):
                        load_xt(xt_sb, g, n)

                beta_sb = small.tile([128, MG, NH], F32, tag="beta")
                GTg = gtp.tile([128, NH, TG], BF16, tag="GT")
                psNB = psMp.tile([128, 2 * MG * 4], F32, tag="psNB")
                pending = []   # deferred attention micro-stages

                def slot():
                    if pending:
                        pending.pop(0)()

                deferred_norm = [None]

                def flush_norm():
                    if deferred_norm[0] is not None:
                        deferred_norm[0]()
                        deferred_norm[0] = None

                for j in range(NH // 4):   # 4-head groups
                    qTg = qkp.tile([128, 4, TG], BF16, tag="qT")
                    kTg = qkp.tile([128, 4, TG], BF16, tag="kT")
                    gvg = vgp.tile([128, MG, 512], BF16, tag="gv")
                    ggg = vgp.tile([128, MG, 512], BF16, tag="gg")
                    rqk = small.tile([128, 2, MG, 4], F32, tag="rqk")
                    psN = psNB.rearrange("p (a m h) -> p a m h", a=2, m=MG)

                    # ---------- q/k: direct-transposed projections ----------
                    for qk, proj in ((0, "q"), (1, "k")):
                        dst = qTg if qk == 0 else kTg
                        if j == 0 and proj in w_pre:
                            w_sb = w_pre.pop(proj)
                        else:
                            w_sb = load_w(qk * 4 + j)
                        for n in range(NT):
                            for hh in range(4):
                                ps = psA.tile([128, 512], F32, tag="psA")
                                for k in range(KH):
                                    nc.tensor.matmul(
                                        ps,
                                        lhsT=w_sb[:, k, hh * 128:(hh + 1) * 128],
                                        rhs=xt_sb[:, k, n * 512:(n + 1) * 512],
                                        start=(k == 0), stop=(k == KH - 1))
                                if has_qk_bias:
                                    src = work.tile([128, 512], F32, tag="qb")
                                    nc.scalar.activation(
                                        src, ps, AF.Identity,
                                        bias=qkb_sb[:, qk * NH + 4 * j + hh:
                                                    qk * NH + 4 * j + hh + 1])
                                else:
                                    src = ps
                                sq = sqp.tile([128, 512], BF16, tag="sq")
                                nc.scalar.activation(sq, src, AF.Square)
                                nc.scalar.copy(
                                    dst[:, hh, n * 512:(n + 1) * 512], src)
                                flush_norm()

                                def mk_norm(sq=sq, qk=qk, hh=hh, n=n):
                                    def emit():
                                        for c in range(4):
                                            m = n * 4 + c
                                            nc.tensor.matmul(
                                                psN[:, qk, m, hh:hh + 1],
                                                lhsT=sq[:, c * 128:(c + 1) * 128],
                                                rhs=ones_sb,
                                                start=True, stop=True)
                                    return emit
                                deferred_norm[0] = mk_norm()
                                slot()

                    # ---- queue S-stages (need only qT/kT; pop during v/og)
                    stage_boxes = {(hh, half): {} for hh in range(4)
                                   for half in range(MG // 4)}

                    def mk_s(qTg=qTg, kTg=kTg, boxes=stage_boxes):
                        out = []
                        for hh in range(4):
                            for half in range(MG // 4):
                                box = boxes[(hh, half)]

                                def s_stage(hh=hh, half=half, box=box):
                                    psS = psSp.tile([128, 4, 128], F32,
                                                    tag="psS")
                                    at = atp.tile([128, 4, 128], BF16,
                                                  tag="at")
                                    for c in range(4):
                                        m = half * 4 + c
                                        msl = slice(m * 128, (m + 1) * 128)
                                        nc.tensor.matmul(
                                            psS[:, c, :],
                                            lhsT=kTg[:, hh, msl],
                                            rhs=qTg[:, hh, msl],
                                            start=True, stop=True)
                                    for c in range(4):
                                        nc.vector.tensor_mul(
                                            at[:, c, :], psS[:, c, :],
                                            mask_sb)
                                    box["at"] = at
                                out.append(s_stage)
                        return out

                    pending.extend(mk_s())

                    # ---- norms -> rqk (and beta on j0); bk = beta * rk ----
                    flush_norm()
                    nln = work.tile([128, 2 * MG * 4], F32, tag="nln")
                    nc.scalar.activation(
                        nln, psN.rearrange("p a m h -> p (a m h)"), AF.Ln)
                    nc.scalar.activation(
                        rqk.rearrange("p a m h -> p (a m h)"), nln,
                        AF.Exp, scale=-0.5)
                    if j == 0:
                        # beta: b-projection (xt fully resident by now;
                        # own PSUM tag so it never serializes against psN)
                        psB = psMp.tile([128, MG, NH], F32, tag="psB")
                        for m in range(MG):
                            for k in range(KH):
                                nc.tensor.matmul(
                                    psB[:, m, :],
                                    lhsT=xt_sb[:, k, m * 128:(m + 1) * 128],
                                    rhs=wb_sb[:, k, :],
                                    start=(k == 0), stop=(k == KH - 1))
                        if has_b_bias:
                            for m in range(MG):
                                nc.vector.tensor_add(
                                    psB[:, m, :], psB[:, m, :], bb_sb)
                        ex_b = work.tile([128, MG * NH], F32, tag="exb")
                        nc.scalar.activation(
                            ex_b, psB.rearrange("p m h -> p (m h)"), AF.Exp)
                        nc.vector.tensor_scalar_add(ex_b, ex_b, 1.0)
                        nc.scalar.activation(
                            beta_sb.rearrange("p m h -> p (m h)"), ex_b, AF.Ln)
                    bk = small.tile([128, MG, 4], F32, tag="bk")
                    nc.vector.tensor_mul(
                        bk, beta_sb[:, :, 4 * j:4 * j + 4], rqk[:, 1, :, :])

                    # ---------- v, og: token-major projections ----------
                    for proj in ("v", "og"):
                        w_sb = load_w((8 if proj == "v" else 12) + j)
                        if has_vog_bias:
                            vb = work.tile([128, 512], F32, tag="vb")
                            off = (0 if proj == "v" else H) + j * 512
                            nc.gpsimd.dma_start(out=vb, in_=bass.AP(
                                tensor=vob.ap().tensor, offset=off,
                                ap=[[0, 128], [1, 512]]))
                        for m in range(MG):
                            ps = psA.tile([128, 512], F32, tag="psA")
                            for k in range(KH):
                                nc.tensor.matmul(
                                    ps,
                                    lhsT=xt_sb[:, k, m * 128:(m + 1) * 128],
                                    rhs=w_sb[:, k, :],
                                    start=(k == 0), stop=(k == KH - 1))
                            if has_vog_bias:
                                nc.vector.tensor_add(ps, ps, vb)
                            if proj == "v":
                                nc.vector.tensor_mul(
                                    gvg[:, m, :].rearrange(
                                        "p (h d) -> p h d", h=4),
                                    ps.rearrange("p (h d) -> p h d", h=4),
                                    bk[:, m, :].to_broadcast([128, 4, 128]))
                            else:
                                # gate = silu(x) * rq (ACT silu table; the
                                # 2 table switches per j are off-critical)
                                sg = work.tile([128, 512], F32, tag="sg")
                                nc.scalar.activation(sg, ps, AF.Silu)
                                nc.vector.tensor_mul(
                                    ggg[:, m, :].rearrange(
                                        "p (h d) -> p h d", h=4),
                                    sg.rearrange("p (h d) -> p h d", h=4),
                                    rqk[:, 0, m, :].to_broadcast(
                                        [128, 4, 128]))
                            slot()

                    # ---------- queue attention O/T stages for this j ----
                    # (S-stages were queued right after the k-phase and pop
                    # during v/og; O/T pop during the NEXT j's projections,
                    # so the pre-o_proj drain is short and pipelines well)
                    def mk_ot(j=j, gvg=gvg, ggg=ggg, GTg=GTg,
                              boxes=stage_boxes):
                        out = []
                        for hh in range(4):
                            for half in range(MG // 4):
                                box = boxes[(hh, half)]

                                def o_stage(hh=hh, half=half, box=box):
                                    at = box["at"]
                                    psO = psA.tile([128, 512], F32,
                                                   tag="psA")
                                    go = gop.tile([128, 4, 128], BF16,
                                                  tag="go")
                                    for c in range(4):
                                        m = half * 4 + c
                                        nc.tensor.matmul(
                                            psO[:, c * 128:(c + 1) * 128],
                                            lhsT=at[:, c, :],
                                            rhs=gvg[:, m,
                                                    hh * 128:(hh + 1) * 128],
                                            start=True, stop=True)
                                    nc.vector.tensor_mul(
                                        go,
                                        psO.rearrange(
                                            "p (c t) -> p c t", c=4),
                                        ggg[:, half * 4:half * 4 + 4,
                                            hh * 128:(hh + 1) * 128])
                                    box["go"] = go

                                def t_stage(hh=hh, half=half, box=box):
                                    go = box["go"]
                                    psT = psTp.tile([128, 4, 128], BF16,
                                                    tag="psT")
                                    for c in range(4):
                                        nc.tensor.transpose(
                                            psT[:, c, :], go[:, c, :], id_sb)
                                    ft = 4 * j + hh
                                    nc.scalar.copy(
                                        GTg[:, ft,
                                            half * 512:(half + 1) * 512],
                                        psT.rearrange("p c t -> p (c t)"))

                                out += [o_stage, t_stage]
                        return out

                    pending.extend(mk_ot())

                # drain attention of the last j before o_proj reads GT
                while pending:
                    pending.pop(0)()

                # ---------------- o_proj for this group ----------------
                for jj in range(H // 512):
                    ow_sb = load_w(16 + jj)
                    if has_o_bias:
                        oby = work.tile([128, 512], F32, tag="oby")
                        nc.gpsimd.dma_start(out=oby, in_=bass.AP(
                            tensor=obd.ap().tensor, offset=jj * 512,
                            ap=[[0, 128], [1, 512]]))
                    for m in range(MG):
                        ps = psA.tile([128, 512], F32, tag="psA")
                        for k in range(KH):
                            nc.tensor.matmul(
                                ps,
                                lhsT=GTg[:, k, m * 128:(m + 1) * 128],
                                rhs=ow_sb[:, k, :],
                                start=(k == 0), stop=(k == KH - 1))
                        if has_o_bias:
                            nc.vector.tensor_add(ps, ps, oby)
                        yt = yp.tile([128, 512], BF16, tag="yt")
                        nc.scalar.copy(yt, ps)
                        nc.sync.dma_start(
                            out=y[g * TG + m * 128:g * TG + (m + 1) * 128,
                                  jj * 512:(jj + 1) * 512],
                            in_=yt)

    nc.compile()
    return nc


def make_host_inputs(hidden_states, q_w, q_b, k_w, k_b, v_w, v_b,
                     a_w, a_b, b_w, b_b, og_w, o_w, o_b, n_cores=N_CORES):
    """Slice/transpose/cast the full inputs into per-core in_maps."""
    B, L, Hh = hidden_states.shape
    assert Hh == H
    X = np.asarray(hidden_states, np.float32).reshape(B * L, H)
    T = (B * L) // n_cores
    assert T % 256 == 0

    # pre-tiled weight blocks: [q0-3|k0-3|v0-3|og0-3|ow0-3], each block b =
    # W.T[:, b*512:(b+1)*512] as [128, KH*512] partition-contiguous
    wcat = np.concatenate(
        [np.asarray(q_w).T, np.asarray(k_w).T, np.asarray(v_w).T,
         np.asarray(og_w).T, np.asarray(o_w).T], axis=1).astype(NPBF)
    wblk = np.ascontiguousarray(
        wcat.reshape(KH, 128, 20, 512).transpose(2, 1, 0, 3).reshape(
            20, 128, KH * 512))
    wbt = np.ascontiguousarray(
        np.asarray(b_w).T.astype(NPBF).reshape(KH, 128, NH).transpose(
            1, 0, 2).reshape(128, KH * NH))
    tri = np.triu(np.ones((CHUNK, CHUNK), np.float32))
    maskt = np.zeros((128, 128), np.float32)
    maskt[:CHUNK, :CHUNK] = tri
    maskt[CHUNK:, CHUNK:] = tri
    ident = np.eye(128, dtype=NPBF)
    onesd = np.ones((128, 1), dtype=NPBF)

    flags = {
        "has_qk_bias": bool(np.any(np.asarray(q_b)) or np.any(np.asarray(k_b))),
        "has_vog_bias": bool(np.any(np.asarray(v_b))),
        "has_b_bias": bool(np.any(np.asarray(b_b))),
        "has_o_bias": bool(np.any(np.asarray(o_b))),
    }
    extras = {}
    if flags["has_qk_bias"]:
        qb = np.asarray(q_b, np.float32).reshape(NH, D).T  # [128, NH]
        kb = np.asarray(k_b, np.float32).reshape(NH, D).T
        extras["qkb"] = np.ascontiguousarray(
            np.concatenate([qb, kb], axis=1))
    if flags["has_vog_bias"]:
        extras["vob"] = np.ascontiguousarray(np.concatenate(
            [np.asarray(v_b, np.float32),
             np.zeros(H, np.float32)]).reshape(1, 2 * H))
    if flags["has_b_bias"]:
        extras["bbd"] = np.ascontiguousarray(
            np.asarray(b_b, np.float32).reshape(1, NH))
    if flags["has_o_bias"]:
        extras["obd"] = np.ascontiguousarray(
            np.asarray(o_b, np.float32).reshape(1, H))

    NB = T // 512
    in_maps = []
    for c in range(n_cores):
        xt_c = X[c * T:(c + 1) * T].T.astype(NPBF)       # [H, T]
        xt3 = np.ascontiguousarray(
            xt_c.reshape(KH, 128, NB, 512).transpose(2, 1, 0, 3).reshape(
                NB, 128, KH * 512))
        m = {"xt3": xt3, "wblk": wblk, "wbt": wbt, "maskt": maskt,
             "ident": ident, "onesd": onesd}
        m.update(extras)
        in_maps.append(m)
    return in_maps, T, flags


_CACHE = {}


def _get_compiled(T, flags):
    """Build + compile once; return a callable mapping in_maps -> per-core
    y arrays (cached jitted executable, same mechanics as
    bass2jax.run_bass_via_pjrt's multi-core path)."""
    key = (T, tuple(sorted(flags.items())))
    if key in _CACHE:
        return _CACHE[key]

    import jax
    from jax.sharding import Mesh, PartitionSpec
    from jax.experimental.shard_map import shard_map
    import concourse.mybir as _mybir
    from concourse import bass2jax

    nc = build_nc(T=T, num_devices=N_CORES, **flags)
    bass2jax.install_neuronx_cc_hook()
    assert nc.dbg_addr is None

    pid_name = (nc.partition_id_tensor.name
                if nc.partition_id_tensor is not None else None)
    in_names, out_names, out_avals = [], [], []
    for alloc in nc.m.functions[0].allocations:
        if not isinstance(alloc, _mybir.MemoryLocationSet):
            continue
        name = alloc.memorylocations[0].name
        if alloc.kind == "ExternalInput":
            if name != pid_name:
                in_names.append(name)
        elif alloc.kind == "ExternalOutput":
            out_names.append(name)
            out_avals.append(jax.core.ShapedArray(
                tuple(alloc.tensor_shape), _mybir.dt.np(alloc.dtype)))
    n_params = len(in_names)
    all_names = in_names + out_names
    if pid_name is not None:
        all_names = all_names + [pid_name]

    def _body(*args):
        operands = list(args)
        if pid_name is not None:
            operands.append(bass2jax.partition_id_tensor())
        outs = bass2jax._bass_exec_p.bind(
            *operands,
            out_avals=tuple(out_avals),
            in_names=tuple(all_names),
            out_names=tuple(out_names),
            lowering_input_output_aliases=(),
            sim_require_finite=True,
            sim_require_nnan=True,
            nc=nc,
        )
        return tuple(outs)

    devices = jax.devices()[:N_CORES]
    mesh = Mesh(np.asarray(devices), ("core",))
    n_outs = len(out_names)
    sharded = jax.jit(
        shard_map(_body, mesh=mesh,
                  in_specs=(PartitionSpec("core"),) * (n_params + n_outs),
                  out_specs=(PartitionSpec("core"),) * n_outs,
                  check_rep=False),
        donate_argnums=tuple(range(n_params, n_params + n_outs)),
        keep_unused=True)

    def run(in_maps):
        concat_in = [
            np.concatenate([np.asarray(in_maps[c][name])
                            for c in range(N_CORES)], axis=0)
            for name in in_names]
        zeros = [np.zeros((N_CORES * a.shape[0],) + a.shape[1:], a.dtype)
                 for a in out_avals]
        out = sharded(*concat_in, *zeros)
        jax.block_until_ready(out)
        ys = np.asarray(out[out_names.index("y")])
        per_core = ys.reshape(N_CORES, -1, ys.shape[-1])
        return per_core

    _CACHE[key] = run
    return run


def kernel(**inputs):
    in_maps, T, flags = make_host_inputs(**inputs)
    run = _get_compiled(T, flags)
    per_core = run(in_maps)
    B, L, Hh = inputs["hidden_states"].shape
    return per_core.reshape(B, L, Hh).astype(np.float32)


# revision 54
# speedup vs baseline: 1.0017x; 1.0017x over previous
"""ChunkwiseDeltaAttention Trainium2 Bass kernel (v2).

Math (per reference):
  q = hs @ q_w.T ; k = ... ; v = ... (heads: 16 x 128; biases are zero
  for the graded inputs -- checked on host, folded paths only built when
  nonzero)
  beta = softplus(hs @ b_w.T) = ln(1 + e^x)
  qn, kn = l2norm per head (the /sqrt(d) pre-scale cancels)
  per 64-chunk: out = tril(qn @ kn^T) @ (beta*v)   (decay==1 on the tri)
  y = (out * silu(hs @ og_w.T)) @ o_w.T

Sharding: token-parallel -- 8 contiguous 2048-token slices, weights
replicated. All matmuls bf16 with f32 PSUM.

Key structure vs v1 (what makes it fast):
  * q/k are projected DIRECTLY transposed (w as stationary, x^T as
    moving) -> [d, tok] tiles with no PE transposes and no token-major
    epilogue chain. Norms are computed with per-tile ones-matmuls
    (sum over d partitions -> [tok,1]) and rsqrt = exp(-0.5*ln(n2)).
    The q-norm folds into the silu gate, the k-norm into beta, so the
    attention math never multiplies by 1/||.|| explicitly.
  * ACT-table usage kept cheap: softplus = ln(1+e^x), rsqrt =
    exp(-0.5*ln); the gate uses the ACT Silu table (square/copy are in
    every set, so only ~2 table switches per head-group, off-critical).
    Never use DVE reciprocal on large tiles (3.4us per [128,512]).
  * Attention (S^T -> mask -> O -> gate) is software-pipelined: its
    PE micro-batches are interleaved between the NEXT projection's
    psum tiles so the PE never waits on the DVE/ACT epilogues.
  * G^T stays in SBUF (no DRAM round-trip before o_proj); o_proj weights
    stream in during attention; y is written bf16.
  * Norm matmuls are emitted one psum-tile late so they never stall PE.
"""

import numpy as np
import ml_dtypes

import concourse.bass as bass
import concourse.mybir as mybir
from concourse import bacc
from concourse.tile import TileContext

BF16 = mybir.dt.bfloat16
F32 = mybir.dt.float32
NPBF = ml_dtypes.bfloat16
AF = mybir.ActivationFunctionType
ALU = mybir.AluOpType

NH = 16      # heads
D = 128      # head dim
CHUNK = 64
H = 2048     # hidden size
N_CORES = 8
KH = H // 128          # hidden k-tiles (16)
COLS = 4 * H + NH      # wcat columns: q|k|v|og|b = 8208
B_OFF = 4 * H


def build_nc(T=2048, num_devices=N_CORES, has_qk_bias=False,
             has_vog_bias=False, has_b_bias=False, has_o_bias=False):
    """Per-core Bass program for a T-token slice."""
    NG = 2 if T >= 2048 else 1
    TG = T // NG           # tokens per group
    MG = TG // 128         # 128-token tiles per group
    NT = TG // 512         # 512-token tiles per group

    nc = bacc.Bacc("TRN2", target_bir_lowering=False, debug=False,
                   num_devices=num_devices)

    # xt3: per-512-token tile-contiguous activations [n, p, (k c)]
    # wblk: pre-tiled weight blocks [q0-3 | k0-3 | v0-3 | og0-3 | ow0-3],
    #       each [128, KH*512] partition-contiguous -> 128 x 16KB descriptors
    NB = T // 512
    xt3 = nc.dram_tensor("xt3", [NB, 128, KH * 512], BF16,
                         kind="ExternalInput")
    wblk = nc.dram_tensor("wblk", [20, 128, KH * 512], BF16,
                          kind="ExternalInput")
    wbt = nc.dram_tensor("wbt", [128, KH * NH], BF16, kind="ExternalInput")
    maskt = nc.dram_tensor("maskt", [128, 128], F32, kind="ExternalInput")
    ident = nc.dram_tensor("ident", [128, 128], BF16, kind="ExternalInput")
    onesd = nc.dram_tensor("onesd", [128, 1], BF16, kind="ExternalInput")
    if has_qk_bias:
        qkb = nc.dram_tensor("qkb", [128, 2 * NH], F32, kind="ExternalInput")
    if has_vog_bias:
        vob = nc.dram_tensor("vob", [1, 2 * H], F32, kind="ExternalInput")
    if has_b_bias:
        bbd = nc.dram_tensor("bbd", [1, NH], F32, kind="ExternalInput")
    if has_o_bias:
        obd = nc.dram_tensor("obd", [1, H], F32, kind="ExternalInput")

    y = nc.dram_tensor("y", [T, H], BF16, kind="ExternalOutput")

    import contextlib
    with TileContext(nc) as tc:
        with contextlib.ExitStack() as _st:
            singles = _st.enter_context(tc.tile_pool(name="singles", bufs=1))
            xtp = _st.enter_context(tc.tile_pool(name="xtp", bufs=1))
            wpool = _st.enter_context(tc.tile_pool(name="wpool", bufs=3))
            qkp = _st.enter_context(tc.tile_pool(name="qkp", bufs=2))
            vgp = _st.enter_context(tc.tile_pool(name="vgp", bufs=2))
            gtp = _st.enter_context(tc.tile_pool(name="gtp", bufs=1))
            sqp = _st.enter_context(tc.tile_pool(name="sqp", bufs=3))
            atp = _st.enter_context(tc.tile_pool(name="atp", bufs=9))
            gop = _st.enter_context(tc.tile_pool(name="gop", bufs=3))
            work = _st.enter_context(tc.tile_pool(name="work", bufs=4))
            yp = _st.enter_context(tc.tile_pool(name="yp", bufs=2))
            small = _st.enter_context(tc.tile_pool(name="small", bufs=2))
            psA = _st.enter_context(
                tc.tile_pool(name="psA", bufs=3, space="PSUM"))
            psSp = _st.enter_context(
                tc.tile_pool(name="psS", bufs=1, space="PSUM"))
            psTp = _st.enter_context(
                tc.tile_pool(name="psT", bufs=2, space="PSUM"))
            psMp = _st.enter_context(
                tc.tile_pool(name="psM", bufs=1, space="PSUM"))
            mask_sb = singles.tile([128, 128], F32)
            nc.sync.dma_start(out=mask_sb, in_=maskt[:, :])
            id_sb = singles.tile([128, 128], BF16)
            nc.sync.dma_start(out=id_sb, in_=ident[:, :])
            ones_sb = singles.tile([128, 1], BF16)
            nc.sync.dma_start(out=ones_sb, in_=onesd[:, :])
            wb_sb = singles.tile([128, KH, NH], BF16)
            nc.sync.dma_start(
                out=wb_sb,
                in_=wbt[:, :].rearrange("p (k c) -> p k c", k=KH))
            if has_qk_bias:
                qkb_sb = singles.tile([128, 2 * NH], F32)
                nc.sync.dma_start(out=qkb_sb, in_=qkb[:, :])
            if has_b_bias:
                bb_sb = singles.tile([128, NH], F32)
                nc.gpsimd.dma_start(out=bb_sb, in_=bass.AP(
                    tensor=bbd.ap().tensor, offset=0, ap=[[0, 128], [1, NH]]))

            def load_w(bi, tag="w"):
                t = wpool.tile([128, KH, 512], BF16, tag=tag)
                nc.sync.dma_start(
                    out=t, in_=wblk[bi, :, :].rearrange(
                        "p (k c) -> p k c", k=KH))
                return t

            def load_xt(xt_sb, g, n):
                nc.sync.dma_start(
                    out=xt_sb[:, :, n * 512:(n + 1) * 512],
                    in_=xt3[g * NT + n, :, :].rearrange(
                        "p (k c) -> p k c", k=KH))

            w_pre = {}
            for g in range(NG):
                xt_sb = xtp.tile([128, KH, TG], BF16, tag="xt")
                if g == 0:
                    # first group: interleave weight + xt streams so the PE
                    # can start early and never starves through j0
                    # interleave quarter-granular w_q / xt streams so the
                    # first psum tile's k=0..3 matmuls start ASAP
                    wq = wpool.tile([128, KH, 512], BF16, tag="w")
                    KQ = KH // 4
                    for qtr in range(4):
                        lo, hi = qtr * KQ, (qtr + 1) * KQ
                        nc.sync.dma_start(
                            out=wq[:, lo:hi, :],
                            in_=wblk[0, :, lo * 512:hi * 512].rearrange(
                                "p (k c) -> p k c", k=KQ))
                        nc.sync.dma_start(
                            out=xt_sb[:, lo:hi, 0:512],
                            in_=xt3[0, :, lo * 512:hi * 512].rearrange(
                                "p (k c) -> p k c", k=KQ))
                    w_pre["q"] = wq
                    for n in range(1, NT):
                        load_xt(xt_sb, g, n)
                    w_pre["k"] = load_w(4)
                else:
                    for n in range(NT):
                        load_xt(xt_sb, g, n)

                beta_sb = small.tile([128, MG, NH], F32, tag="beta")
                GTg = gtp.tile([128, NH, TG], BF16, tag="GT")
                psNB = psMp.tile([128, 2 * MG * 4], F32, tag="psNB")
                pending = []   # deferred attention micro-stages

                def slot():
                    if pending:
                        pending.pop(0)()

                deferred_norm = [None]

                def flush_norm():
                    if deferred_norm[0] is not None:
                        deferred_norm[0]()
                        deferred_norm[0] = None

                for j in range(NH // 4):   # 4-head groups
                    qTg = qkp.tile([128, 4, TG], BF16, tag="qT")
                    kTg = qkp.tile([128, 4, TG], BF16, tag="kT")
                    gvg = vgp.tile([128, MG, 512], BF16, tag="gv")
                    ggg = vgp.tile([128, MG, 512], BF16, tag="gg")
                    rqk = small.tile([128, 2, MG, 4], F32, tag="rqk")
                    psN = psNB.rearrange("p (a m h) -> p a m h", a=2, m=MG)

                    # ---------- q/k: direct-transposed projections ----------
                    for qk, proj in ((0, "q"), (1, "k")):
                        dst = qTg if qk == 0 else kTg
                        if j == 0 and proj in w_pre:
                            w_sb = w_pre.pop(proj)
                        else:
                            w_sb = load_w(qk * 4 + j)
                        for n in range(NT):
                            for hh in range(4):
                                ps = psA.tile([128, 512], F32, tag="psA")
                                for k in range(KH):
                                    nc.tensor.matmul(
                                        ps,
                                        lhsT=w_sb[:, k, hh * 128:(hh + 1) * 128],
                                        rhs=xt_sb[:, k, n * 512:(n + 1) * 512],
                                        start=(k == 0), stop=(k == KH - 1))
                                if has_qk_bias:
                                    src = work.tile([128, 512], F32, tag="qb")
                                    nc.scalar.activation(
                                        src, ps, AF.Identity,
                                        bias=qkb_sb[:, qk * NH + 4 * j + hh:
                                                    qk * NH + 4 * j + hh + 1])
                                else:
                                    src = ps
                                dsl = dst[:, hh, n * 512:(n + 1) * 512]
                                nc.scalar.copy(dsl, src)
                                # square the bf16 copy on DVE (SBUF x SBUF
                                # is legal there); keeps ACT to one op/tile
                                sq = sqp.tile([128, 512], BF16, tag="sq")
                                nc.vector.tensor_mul(sq, dsl, dsl)
                                flush_norm()

                                def mk_norm(sq=sq, qk=qk, hh=hh, n=n):
                                    def emit():
                                        for c in range(4):
                                            m = n * 4 + c
                                            nc.tensor.matmul(
                                                psN[:, qk, m, hh:hh + 1],
                                                lhsT=sq[:, c * 128:(c + 1) * 128],
                                                rhs=ones_sb,
                                                start=True, stop=True)
                                    return emit
                                deferred_norm[0] = mk_norm()
                                slot()

                    # ---- queue S-stages (need only qT/kT; pop during v/og)
                    stage_boxes = {(hh, half): {} for hh in range(4)
                                   for half in range(MG // 4)}

                    def mk_s(qTg=qTg, kTg=kTg, boxes=stage_boxes):
                        out = []
                        for hh in range(4):
                            for half in range(MG // 4):
                                box = boxes[(hh, half)]

                                def s_stage(hh=hh, half=half, box=box):
                                    psS = psSp.tile([128, 4, 128], F32,
                                                    tag="psS")
                                    at = atp.tile([128, 4, 128], BF16,
                                                  tag="at")
                                    for c in range(4):
                                        m = half * 4 + c
                                        msl = slice(m * 128, (m + 1) * 128)
                                        nc.tensor.matmul(
                                            psS[:, c, :],
                                            lhsT=kTg[:, hh, msl],
                                            rhs=qTg[:, hh, msl],
                                            start=True, stop=True)
                                    for c in range(4):
                                        nc.vector.tensor_mul(
                                            at[:, c, :], psS[:, c, :],
                                            mask_sb)
                                    box["at"] = at
                                out.append(s_stage)
                        return out

                    pending.extend(mk_s())

                    # ---- norms -> rqk (and beta on j0); bk = beta * rk ----
                    flush_norm()
                    nln = work.tile([128, 2 * MG * 4], F32, tag="nln")
                    nc.scalar.activation(
                        nln, psN.rearrange("p a m h -> p (a m h)"), AF.Ln)
                    nc.scalar.activation(
                        rqk.rearrange("p a m h -> p (a m h)"), nln,
                        AF.Exp, scale=-0.5)
                    if j == 0:
                        # beta: b-projection (xt fully resident by now;
                        # own PSUM tag so it never serializes against psN)
                        psB = psMp.tile([128, MG, NH], F32, tag="psB")
                        for m in range(MG):
                            for k in range(KH):
                                nc.tensor.matmul(
                                    psB[:, m, :],
                                    lhsT=xt_sb[:, k, m * 128:(m + 1) * 128],
                                    rhs=wb_sb[:, k, :],
                                    start=(k == 0), stop=(k == KH - 1))
                        if has_b_bias:
                            for m in range(MG):
                                nc.vector.tensor_add(
                                    psB[:, m, :], psB[:, m, :], bb_sb)
                        ex_b = work.tile([128, MG * NH], F32, tag="exb")
                        nc.scalar.activation(
                            ex_b, psB.rearrange("p m h -> p (m h)"), AF.Exp)
                        nc.vector.tensor_scalar_add(ex_b, ex_b, 1.0)
                        nc.scalar.activation(
                            beta_sb.rearrange("p m h -> p (m h)"), ex_b, AF.Ln)
                    bk = small.tile([128, MG, 4], F32, tag="bk")
                    nc.vector.tensor_mul(
                        bk, beta_sb[:, :, 4 * j:4 * j + 4], rqk[:, 1, :, :])

                    # ---------- v, og: token-major projections ----------
                    for proj in ("v", "og"):
                        w_sb = load_w((8 if proj == "v" else 12) + j)
                        if has_vog_bias:
                            vb = work.tile([128, 512], F32, tag="vb")
                            off = (0 if proj == "v" else H) + j * 512
                            nc.gpsimd.dma_start(out=vb, in_=bass.AP(
                                tensor=vob.ap().tensor, offset=off,
                                ap=[[0, 128], [1, 512]]))
                        for m in range(MG):
                            ps = psA.tile([128, 512], F32, tag="psA")
                            for k in range(KH):
                                nc.tensor.matmul(
                                    ps,
                                    lhsT=xt_sb[:, k, m * 128:(m + 1) * 128],
                                    rhs=w_sb[:, k, :],
                                    start=(k == 0), stop=(k == KH - 1))
                            if has_vog_bias:
                                nc.vector.tensor_add(ps, ps, vb)
                            if proj == "v":
                                nc.vector.tensor_mul(
                                    gvg[:, m, :].rearrange(
                                        "p (h d) -> p h d", h=4),
                                    ps.rearrange("p (h d) -> p h d", h=4),
                                    bk[:, m, :].to_broadcast([128, 4, 128]))
                            else:
                                # gate = silu(x) * rq (ACT silu table; the
                                # 2 table switches per j are off-critical)
                                sg = work.tile([128, 512], F32, tag="sg")
                                nc.scalar.activation(sg, ps, AF.Silu)
                                nc.vector.tensor_mul(
                                    ggg[:, m, :].rearrange(
                                        "p (h d) -> p h d", h=4),
                                    sg.rearrange("p (h d) -> p h d", h=4),
                                    rqk[:, 0, m, :].to_broadcast(
                                        [128, 4, 128]))
                            slot()

                    # ---------- queue attention O/T stages for this j ----
                    # (S-stages were queued right after the k-phase and pop
                    # during v/og; O/T pop during the NEXT j's projections,
                    # so the pre-o_proj drain is short and pipelines well)
                    def mk_ot(j=j, gvg=gvg, ggg=ggg, GTg=GTg,
                              boxes=stage_boxes):
                        out = []
                        for hh in range(4):
                            for half in range(MG // 4):
                                box = boxes[(hh, half)]

                                def o_stage(hh=hh, half=half, box=box):
                                    at = box["at"]
                                    psO = psA.tile([128, 512], F32,
                                                   tag="psA")
                                    go = gop.tile([128, 4, 128], BF16,
                                                  tag="go")
                                    for c in range(4):
                                        m = half * 4 + c
                                        nc.tensor.matmul(
                                            psO[:, c * 128:(c + 1) * 128],
                                            lhsT=at[:, c, :],
                                            rhs=gvg[:, m,
                                                    hh * 128:(hh + 1) * 128],
                                            start=True, stop=True)
                                    nc.vector.tensor_mul(
                                        go,
                                        psO.rearrange(
                                            "p (c t) -> p c t", c=4),
                                        ggg[:, half * 4:half * 4 + 4,
                                            hh * 128:(hh + 1) * 128])
                                    box["go"] = go

                                def t_stage(hh=hh, half=half, box=box):
                                    go = box["go"]
                                    psT = psTp.tile([128, 4, 128], BF16,
                                                    tag="psT")
                                    for c in range(4):
                                        nc.tensor.transpose(
                                            psT[:, c, :], go[:, c, :], id_sb)
                                    ft = 4 * j + hh
                                    nc.scalar.copy(
                                        GTg[:, ft,
                                            half * 512:(half + 1) * 512],
                                        psT.rearrange("p c t -> p (c t)"))

                                out += [o_stage, t_stage]
                        return out

                    pending.extend(mk_ot())

                # drain attention of the last j before o_proj reads GT
                while pending:
                    pending.pop(0)()

                # ---------------- o_proj for this group ----------------
                for jj in range(H // 512):
                    ow_sb = load_w(16 + jj)
                    if has_o_bias:
                        oby = work.tile([128, 512], F32, tag="oby")
                        nc.gpsimd.dma_start(out=oby, in_=bass.AP(
                            tensor=obd.ap().tensor, offset=jj * 512,
                            ap=[[0, 128], [1, 512]]))
                    for m in range(MG):
                        ps = psA.tile([128, 512], F32, tag="psA")
                        for k in range(KH):
                            nc.tensor.matmul(
                                ps,
                                lhsT=GTg[:, k, m * 128:(m + 1) * 128],
                                rhs=ow_sb[:, k, :],
                                start=(k == 0), stop=(k == KH - 1))
                        if has_o_bias:
                            nc.vector.tensor_add(ps, ps, oby)
                        yt = yp.tile([128, 512], BF16, tag="yt")
                        nc.scalar.copy(yt, ps)
                        nc.sync.dma_start(
                            out=y[g * TG + m * 128:g * TG + (m + 1) * 128,
                                  jj * 512:(jj + 1) * 512],
                            in_=yt)

    nc.compile()
    return nc


def make_host_inputs(hidden_states, q_w, q_b, k_w, k_b, v_w, v_b,
                     a_w, a_b, b_w, b_b, og_w, o_w, o_b, n_cores=N_CORES):
    """Slice/transpose/cast the full inputs into per-core in_maps."""
    B, L, Hh = hidden_states.shape
    assert Hh == H
    X = np.asarray(hidden_states, np.float32).reshape(B * L, H)
    T = (B * L) // n_cores
    assert T % 256 == 0

    # pre-tiled weight blocks: [q0-3|k0-3|v0-3|og0-3|ow0-3], each block b =
    # W.T[:, b*512:(b+1)*512] as [128, KH*512] partition-contiguous
    wcat = np.concatenate(
        [np.asarray(q_w).T, np.asarray(k_w).T, np.asarray(v_w).T,
         np.asarray(og_w).T, np.asarray(o_w).T], axis=1).astype(NPBF)
    wblk = np.ascontiguousarray(
        wcat.reshape(KH, 128, 20, 512).transpose(2, 1, 0, 3).reshape(
            20, 128, KH * 512))
    wbt = np.ascontiguousarray(
        np.asarray(b_w).T.astype(NPBF).reshape(KH, 128, NH).transpose(
            1, 0, 2).reshape(128, KH * NH))
    tri = np.triu(np.ones((CHUNK, CHUNK), np.float32))
    maskt = np.zeros((128, 128), np.float32)
    maskt[:CHUNK, :CHUNK] = tri
    maskt[CHUNK:, CHUNK:] = tri
    ident = np.eye(128, dtype=NPBF)
    onesd = np.ones((128, 1), dtype=NPBF)

    flags = {
        "has_qk_bias": bool(np.any(np.asarray(q_b)) or np.any(np.asarray(k_b))),
        "has_vog_bias": bool(np.any(np.asarray(v_b))),
        "has_b_bias": bool(np.any(np.asarray(b_b))),
        "has_o_bias": bool(np.any(np.asarray(o_b))),
    }
    extras = {}
    if flags["has_qk_bias"]:
        qb = np.asarray(q_b, np.float32).reshape(NH, D).T  # [128, NH]
        kb = np.asarray(k_b, np.float32).reshape(NH, D).T
        extras["qkb"] = np.ascontiguousarray(
            np.concatenate([qb, kb], axis=1))
    if flags["has_vog_bias"]:
        extras["vob"] = np.ascontiguousarray(np.concatenate(
            [np.asarray(v_b, np.float32),
             np.zeros(H, np.float32)]).reshape(1, 2 * H))
    if flags["has_b_bias"]:
        extras["bbd"] = np.ascontiguousarray(
            np.asarray(b_b, np.float32).reshape(1, NH))
    if flags["has_o_bias"]:
        extras["obd"] = np.ascontiguousarray(
            np.asarray(o_b, np.float32).reshape(1, H))

    NB = T // 512
    in_maps = []
    for c in range(n_cores):
        xt_c = X[c * T:(c + 1) * T].T.astype(NPBF)       # [H, T]
        xt3 = np.ascontiguousarray(
            xt_c.reshape(KH, 128, NB, 512).transpose(2, 1, 0, 3).reshape(
                NB, 128, KH * 512))
        m = {"xt3": xt3, "wblk": wblk, "wbt": wbt, "maskt": maskt,
             "ident": ident, "onesd": onesd}
        m.update(extras)
        in_maps.append(m)
    return in_maps, T, flags


_CACHE = {}


def _get_compiled(T, flags):
    """Build + compile once; return a callable mapping in_maps -> per-core
    y arrays (cached jitted executable, same mechanics as
    bass2jax.run_bass_via_pjrt's multi-core path)."""
    key = (T, tuple(sorted(flags.items())))
    if key in _CACHE:
        return _CACHE[key]

    import jax
    from jax.sharding import Mesh, PartitionSpec
    from jax.experimental.shard_map import shard_map
    import concourse.mybir as _mybir
    from concourse import bass2jax

    nc = build_nc(T=T, num_devices=N_CORES, **flags)
    bass2jax.install_neuronx_cc_hook()
    assert nc.dbg_addr is None

    pid_name = (nc.partition_id_tensor.name
                if nc.partition_id_tensor is not None else None)
    in_names, out_names, out_avals = [], [], []
    for alloc in nc.m.functions[0].allocations:
        if not isinstance(alloc, _mybir.MemoryLocationSet):
            continue
        name = alloc.memorylocations[0].name
        if alloc.kind == "ExternalInput":
            if name != pid_name:
                in_names.append(name)
        elif alloc.kind == "ExternalOutput":
            out_names.append(name)
            out_avals.append(jax.core.ShapedArray(
                tuple(alloc.tensor_shape), _mybir.dt.np(alloc.dtype)))
    n_params = len(in_names)
    all_names = in_names + out_names
    if pid_name is not None:
        all_names = all_names + [pid_name]

    def _body(*args):
        operands = list(args)
        if pid_name is not None:
            operands.append(bass2jax.partition_id_tensor())
        outs = bass2jax._bass_exec_p.bind(
            *operands,
            out_avals=tuple(out_avals),
            in_names=tuple(all_names),
            out_names=tuple(out_names),
            lowering_input_output_aliases=(),
            sim_require_finite=True,
            sim_require_nnan=True,
            nc=nc,
        )
        return tuple(outs)

    devices = jax.devices()[:N_CORES]
    mesh = Mesh(np.asarray(devices), ("core",))
    n_outs = len(out_names)
    sharded = jax.jit(
        shard_map(_body, mesh=mesh,
                  in_specs=(PartitionSpec("core"),) * (n_params + n_outs),
                  out_specs=(PartitionSpec("core"),) * n_outs,
                  check_rep=False),
        donate_argnums=tuple(range(n_params, n_params + n_outs)),
        keep_unused=True)

    def run(in_maps):
        concat_in = [
            np.concatenate([np.asarray(in_maps[c][name])
                            for c in range(N_CORES)], axis=0)
            for name in in_names]
        zeros = [np.zeros((N_CORES * a.shape[0],) + a.shape[1:], a.dtype)
                 for a in out_avals]
        out = sharded(*concat_in, *zeros)
        jax.block_until_ready(out)
        ys = np.asarray(out[out_names.index("y")])
        per_core = ys.reshape(N_CORES, -1, ys.shape[-1])
        return per_core

    _CACHE[key] = run
    return run


def kernel(**inputs):
    in_maps, T, flags = make_host_inputs(**inputs)
    run = _get_compiled(T, flags)
    per_core = run(in_maps)
    B, L, Hh = inputs["hidden_states"].shape
    return per_core.reshape(B, L, Hh).astype(np.float32)
